# revision 37
# baseline (speedup 1.0000x reference)
"""GQA causal-attention prefill kernel for 8 TRN2 NeuronCores.

Sharding (zero cross-core comm): 8 KV heads -> 1 per core, each with its 4
GQA query heads. Per core: Wq slice [512,2560], Wk/Wv slice [128,2560], Wo
column-slice [2560,512]. Each core computes a full [2048,2560] partial of
the output projection; the host sums the 8 partials.

Per-core math (all matmuls in bf16, f32 PSUM accumulation):
  proj:  q/k/v seq-major via stationary=hsT tiles, moving=W^T slabs
  norm+rope on seq-major tiles (DVE), fold norm weights into cos/sin tables
  rstd via exp(-0.5*ln(var+eps)) so ACT stays in one table (exp/ln/square)
  PE-transpose roped q,k -> feature-major qT,kT (bf16 PSUM)
  S^T = kT_tile.T @ qT  (lower-triangle blocks only, lookahead-2 issue)
  P^T = exp(scale*S^T) (ACT); diagonal blocks zeroed by binary-mask multiply
  rowsum via ones matmul; AV: attnT = v.T @ P^T; normalize via approx recip
  out_partial = attnT.T @ WoT, interleaved per chunk so output DMA spreads
  across the whole kernel instead of crunching at the tail.

DMAs are issued in need-order (first weight chunks + first hsT slab first)
so the first projection matmul starts ~2us in. Evictions run on the
otherwise-idle GpSimd engine.
"""

import ml_dtypes
import numpy as np

import concourse.bass as bass
import concourse.mybir as mybir
import concourse.tile as tile
from concourse import bacc
from concourse.bass_utils import run_bass_kernel_spmd

P = 128
S = 2048
H = 2560
NS = S // P          # 16 s-tiles
NHT = H // P         # 20 hidden tiles
NH = 4               # q heads per core
DQ = NH * P          # 512
DKV = 2 * P          # 256 (k|v)
NCH = 4              # sq chunks of 512
CW = 512
NJC = H // CW        # 5 output col chunks
SCALE = float(P) ** -0.5
EPS = 1e-6

F32 = mybir.dt.float32
BF16 = mybir.dt.bfloat16

# rsqrt(var) ~ deg-3 poly on var in [0.6, 2.9] + 1 Newton step (max rel err
# ~1e-3); keeps rstd off the ACT engine so ACT never leaves its exp table.
RSQ_C3 = -0.06104849
RSQ_C2 = 0.44709427
RSQ_C1 = -1.21807019
RSQ_C0 = 1.84091155

_CACHE = {}


def _build():
    nc = bacc.Bacc("TRN2", target_bir_lowering=False)

    hst = nc.declare_dram_parameter("hst", [NS, P, NHT * P], BF16, isOutput=False)
    wqt = nc.declare_dram_parameter("wqt", [P, NHT * DQ], BF16, isOutput=False)
    wkvt = nc.declare_dram_parameter("wkvt", [P, NHT * DKV], BF16, isOutput=False)
    wot = nc.declare_dram_parameter("wot", [P, NH * H], BF16, isOutput=False)
    tab = nc.declare_dram_parameter("tab", [NS, P, 4 * P], BF16, isOutput=False)
    maskb = nc.declare_dram_parameter("maskb", [P, P], BF16, isOutput=False)
    ident = nc.declare_dram_parameter("ident", [P, P], BF16, isOutput=False)
    onessq = nc.declare_dram_parameter("onessq", [P, P], BF16, isOutput=False)
    out = nc.declare_dram_parameter("out", [S, H], BF16, isOutput=True)

    with tile.TileContext(nc) as tc:
        with (
            tc.tile_pool(name="wq", bufs=1) as wq_pool,        # WqT slab
            tc.tile_pool(name="wkv", bufs=1) as wkv_pool,      # WkvT slab
            tc.tile_pool(name="wow", bufs=1) as wow_pool,      # WoT slab
            tc.tile_pool(name="qa", bufs=5) as qa_pool,        # qT_g / attnT_g
            tc.tile_pool(name="kv", bufs=1) as kv_pool,        # kT + v
            tc.tile_pool(name="big", bufs=6) as big_pool,      # hsT slabs
            tc.tile_pool(name="os", bufs=3) as os_pool,        # out staging
            tc.tile_pool(name="tab", bufs=4) as tab_pool,      # table blocks
            tc.tile_pool(name="wk", bufs=9) as wk_pool,        # rope scratch
            tc.tile_pool(name="qw", bufs=4) as qw_pool,        # roped q/k
            tc.tile_pool(name="pt", bufs=10) as pt_pool,       # P^T tiles
            tc.tile_pool(name="sm", bufs=8) as sm_pool,        # small stats
            tc.tile_pool(name="cst", bufs=1) as cst_pool,      # consts
            tc.tile_pool(name="ps", bufs=7, space="PSUM") as ps_pool,
            tc.tile_pool(name="tp", bufs=1, space="PSUM") as tp_pool,
        ):
            slabs, tabts = {}, {}

            def load_inputs(i, split=4):
                if i in slabs or i >= NS:
                    return
                # split across DMA queues: a single queue only gets ~1/16
                # of HBM bandwidth when all queues are busy
                slabs[i] = big_pool.tile([P, NHT * P], BF16, tag="big",
                                         name=f"slab_{i}")
                qn = NHT * P // split
                for q4 in range(split):
                    nc.sync.dma_start(
                        slabs[i][:, q4 * qn:(q4 + 1) * qn],
                        hst.ap()[i][:, q4 * qn:(q4 + 1) * qn],
                    )
                tabts[i] = tab_pool.tile([P, 4 * P], BF16, tag="tab",
                                         name=f"tabt_{i}")
                nc.sync.dma_start(tabts[i][:], tab.ap()[i])

            wqt_sb = wq_pool.tile([P, NHT * DQ], BF16, tag="w")
            wkvt_sb = wkv_pool.tile([P, NHT * DKV], BF16, tag="w")
            wot_sb = wow_pool.tile([P, NH * H], BF16, tag="w")

            # ---- DMAs in need-order: first wq chunks + slab0 pieces, then
            # the rest of the weights, early slabs, and WoT ----
            slabs[0] = big_pool.tile([P, NHT * P], BF16, tag="big",
                                     name="slab_0")
            # absolute minimum for the first (kv) matmul: WkvT t=0 + slab0
            # t=0; the small WkvT streams in fully while WqT follows
            hkv = NHT * DKV // 4
            nc.sync.dma_start(wkvt_sb[:, 0:hkv], wkvt.ap()[:, 0:hkv])
            nc.sync.dma_start(slabs[0][:, 0:P], hst.ap()[0][:, 0:P])
            nc.sync.dma_start(slabs[0][:, P:2 * P], hst.ap()[0][:, P:2 * P])
            for c4 in range(1, 4):
                nc.sync.dma_start(wkvt_sb[:, c4 * hkv:(c4 + 1) * hkv],
                                  wkvt.ap()[:, c4 * hkv:(c4 + 1) * hkv])
            CC = 2 * DQ  # two t-chunks of WqT per DMA
            sq = NHT * P // 4
            nc.sync.dma_start(slabs[0][:, 2 * P:sq], hst.ap()[0][:, 2 * P:sq])
            for cc in range(10):
                nc.sync.dma_start(wqt_sb[:, cc * CC:(cc + 1) * CC],
                                  wqt.ap()[:, cc * CC:(cc + 1) * CC])
                if cc < 3:
                    nc.sync.dma_start(
                        slabs[0][:, (cc + 1) * sq:(cc + 2) * sq],
                        hst.ap()[0][:, (cc + 1) * sq:(cc + 2) * sq])
                elif cc == 3:
                    tabts[0] = tab_pool.tile([P, 4 * P], BF16, tag="tab",
                                             name="tabt_0")
                    nc.sync.dma_start(tabts[0][:], tab.ap()[0])
                elif cc == 4:
                    # consts (needed from the first transpose/attn on)
                    maskb_sb = cst_pool.tile([P, P], BF16, tag="maskb")
                    nc.sync.dma_start(maskb_sb[:], maskb.ap())
                    ident_sb = cst_pool.tile([P, P], BF16, tag="ident")
                    nc.sync.dma_start(ident_sb[:], ident.ap())
                    onessq_sb = cst_pool.tile([P, P], BF16, tag="onessq")
                    nc.sync.dma_start(onessq_sb[:], onessq.ap())
                elif cc == 5:
                    load_inputs(1)
            load_inputs(2)
            load_inputs(3)
            hwo = NH * H // 4
            for cc in range(4):
                nc.sync.dma_start(wot_sb[:, cc * hwo:(cc + 1) * hwo],
                                  wot.ap()[:, cc * hwo:(cc + 1) * hwo])

            # persistent attention operands
            kT = kv_pool.tile([P, S], BF16, tag="kt")          # [d, t]
            v_sb = kv_pool.tile([P, NS, P], BF16, tag="v")     # [t, tile, d]

            qT = [None] * NCH
            attnT = [None] * NCH
            pending_tp = [None]  # deferred transpose of the previous tile

            def flush_tp():
                if pending_tp[0] is None:
                    return
                i, g, r, q_ro, k_ro = pending_tp[0]
                pending_tp[0] = None
                tp = tp_pool.tile([P, 5 * P], BF16, tag="tp", name=f"tp_{i}")
                for h in range(NH):
                    nc.tensor.transpose(
                        tp[:, h * P:(h + 1) * P], q_ro[:, h * P:(h + 1) * P],
                        ident_sb[:],
                    )
                nc.tensor.transpose(tp[:, DQ:DQ + P], k_ro[:], ident_sb[:])
                if qT[g] is None:
                    qT[g] = qa_pool.tile([P, NCH, NH, P], BF16, tag="qa",
                                         name=f"qT_{g}")
                nc.vector.tensor_copy(
                    qT[g][:, r, :, :].rearrange("p h d -> p (h d)"),
                    tp[:, 0:DQ],
                )
                nc.vector.tensor_copy(kT[:, i * P:(i + 1) * P], tp[:, DQ:DQ + P])

            proj_ps = {}

            def proj_mm(i):
                load_inputs(i + 2)
                load_inputs(i + 3)
                slab = slabs.pop(i)

                q_ps = ps_pool.tile([P, DQ], F32, tag="mm")
                kv_ps = ps_pool.tile([P, DKV], F32, tag="mm")
                # kv first for the DMA-starved opening tiles (WkvT is small
                # and lands before the WqT stream completes)
                passes = [(kv_ps, wkvt_sb, DKV), (q_ps, wqt_sb, DQ)]
                if i >= 3:
                    passes.reverse()
                for pi, (ps, wsb, dw) in enumerate(passes):
                    for t in range(NHT):
                        nc.tensor.matmul(
                            ps[:], slab[:, t * P:(t + 1) * P],
                            wsb[:, t * dw:(t + 1) * dw],
                            start=(t == 0), stop=(t == NHT - 1),
                        )
                    if pi == 0:
                        # the previous tile's transposes land between the
                        # two projection passes: its rope chain (DVE) hides
                        # under pass 1, and the qT/kT copies (DVE) hide
                        # under pass 2 — so a directly-following attention
                        # phase finds qT ready
                        flush_tp()
                proj_ps[i] = (q_ps, kv_ps)

            def proj_post(i):
                g, r = i // NCH, i % NCH
                q_ps, kv_ps = proj_ps.pop(i)
                tabt = tabts.pop(i)
                cq_t, sq_t = tabt[:, 0:P], tabt[:, P:2 * P]
                ck_t, sk_t = tabt[:, 2 * P:3 * P], tabt[:, 3 * P:4 * P]

                # v evict (f32 psum -> bf16); gpsimd can't read PSUM
                nc.scalar.activation(v_sb[:, i, :], kv_ps[:, P:DKV],
                                     mybir.ActivationFunctionType.Copy)

                # ---- rms-norm stats (ACT squares, DVE rsqrt poly) ----
                q2 = wk_pool.tile([P, DQ], F32, tag="wk", name=f"q2_{i}")
                nc.scalar.activation(
                    q2[:], q_ps[:, 0:DQ],
                    mybir.ActivationFunctionType.Square,
                )
                ss = sm_pool.tile([P, NH + 1], F32, tag="ssq")
                nc.vector.tensor_reduce(
                    ss[:, 0:NH], q2[:].rearrange("p (h d) -> p h d", h=NH),
                    mybir.AxisListType.X, mybir.AluOpType.add,
                )
                junk = sm_pool.tile([P, P], F32, tag="junk")
                nc.scalar.activation(
                    junk[:], kv_ps[:, 0:P],
                    mybir.ActivationFunctionType.Square,
                    accum_out=ss[:, NH:NH + 1],
                )
                # rstd = rsqrt(ss/P) via poly+Newton on DVE (keeps ACT in
                # its exp table; Sqrt/Ln would force 1.3us table reloads).
                # Poly in raw ss: coefficients pre-divided by powers of P.
                NW = NH + 1
                h1 = sm_pool.tile([P, NW], F32, tag="h1")
                nc.vector.tensor_scalar(
                    h1[:], ss[:], RSQ_C3 / P ** 3, RSQ_C2 / P ** 2,
                    mybir.AluOpType.mult, mybir.AluOpType.add,
                )
                nc.vector.tensor_tensor(h1[:], h1[:], ss[:],
                                        mybir.AluOpType.mult)
                nc.vector.tensor_scalar_add(h1[:], h1[:], RSQ_C1 / P)
                y0 = sm_pool.tile([P, NW], F32, tag="y0")
                nc.vector.tensor_tensor(y0[:], h1[:], ss[:],
                                        mybir.AluOpType.mult)
                nc.vector.tensor_scalar_add(y0[:], y0[:], RSQ_C0)
                # Newton: rstd = y0 * (1.5 - (ss/(2P))*y0^2)
                t1 = sm_pool.tile([P, NW], F32, tag="t1")
                nc.vector.tensor_tensor(t1[:], y0[:], y0[:],
                                        mybir.AluOpType.mult)
                nc.vector.tensor_tensor(t1[:], t1[:], ss[:],
                                        mybir.AluOpType.mult)
                nc.vector.tensor_scalar(
                    t1[:], t1[:], -0.5 / P, 1.5,
                    mybir.AluOpType.mult, mybir.AluOpType.add,
                )
                rstd = sm_pool.tile([P, NW], F32, tag="rsq")
                nc.vector.tensor_tensor(rstd[:], y0[:], t1[:],
                                        mybir.AluOpType.mult)
                rstd_q, rstd_k = rstd[:, 0:NH], rstd[:, NH:NH + 1]

                # ---- fused norm-scale + rope (DVE, bf16 after first mult) ----
                def rope(ps_slice, nh, rstd, cos_t, sin_t, nm):
                    w = nh * P
                    qn = wk_pool.tile([P, w], BF16, tag="wk", name=f"qn_{nm}_{i}")
                    q3 = qn[:].rearrange("p (h d) -> p h d", h=nh)
                    nc.vector.tensor_tensor(
                        q3, ps_slice.rearrange("p (h d) -> p h d", h=nh),
                        rstd[:, :, None].broadcast_to([P, nh, P]),
                        mybir.AluOpType.mult,
                    )
                    r1 = wk_pool.tile([P, w], BF16, tag="wk", name=f"r1_{nm}_{i}")
                    nc.vector.tensor_tensor(
                        r1[:].rearrange("p (h d) -> p h d", h=nh), q3,
                        cos_t[:, None, :].broadcast_to([P, nh, P]),
                        mybir.AluOpType.mult,
                    )
                    r2 = wk_pool.tile([P, w], BF16, tag="wk", name=f"r2_{nm}_{i}")
                    r23 = r2[:].rearrange("p (h d) -> p h d", h=nh)
                    nc.vector.tensor_tensor(
                        r23[:, :, 0:64], q3[:, :, 64:P],
                        sin_t[:, None, 0:64].broadcast_to([P, nh, 64]),
                        mybir.AluOpType.mult,
                    )
                    nc.vector.tensor_tensor(
                        r23[:, :, 64:P], q3[:, :, 0:64],
                        sin_t[:, None, 64:P].broadcast_to([P, nh, 64]),
                        mybir.AluOpType.mult,
                    )
                    ro = qw_pool.tile([P, w], BF16, tag="qw", name=f"ro_{nm}_{i}")
                    nc.vector.tensor_tensor(
                        ro[:], r1[:], r2[:], mybir.AluOpType.add,
                    )
                    return ro

                q_ro = rope(q_ps[:, 0:DQ], NH, rstd_q, cq_t, sq_t, "q")
                k_ro = rope(kv_ps[:, 0:P], 1, rstd_k, ck_t, sk_t, "k")
                # transposes deferred to the next tile's matmul shadow
                pending_tp[0] = (i, g, r, q_ro, k_ro)

            def s_mm(g, h, j, sts):
                r0 = max(0, j - 4 * g)
                w = CW - r0 * P
                st = ps_pool.tile([P, CW], F32, tag="mm",
                                  name=f"st_{g}_{h}_{j}")
                nc.tensor.matmul(
                    st[:, 0:w],
                    kT[:, j * P:(j + 1) * P],
                    qT[g][:, r0:NCH, h, :],
                )
                sts[(h, j)] = st

            def attn_phase(g, hs, warm=None, look=2):
                """Attention units (heads hs) as one flat S-matmul stream
                with cross-unit lookahead, so the PE never drains at the
                unit boundary waiting for the first exp. `warm` carries S
                tiles pre-issued by the previous phase."""
                njt = 4 * g + 4  # t-tiles 0..4g+3
                seq = [(h, j) for h in hs for j in range(njt)]
                sts = warm if warm is not None else {}
                acc = {}  # h -> (av_ps, rb_ps)

                LOOK = look
                for k in range(min(LOOK, len(seq))):
                    if seq[k] not in sts:
                        s_mm(g, seq[k][0], seq[k][1], sts)
                for k, (h, j) in enumerate(seq):
                    if k + LOOK < len(seq) and seq[k + LOOK] not in sts:
                        h2, j2 = seq[k + LOOK]
                        s_mm(g, h2, j2, sts)
                    if j == 0:
                        acc[h] = (
                            ps_pool.tile([P, CW], F32, tag="mm",
                                         name=f"av_{g}_{h}"),
                            ps_pool.tile([P, CW], F32, tag="mm",
                                         name=f"rb_{g}_{h}"),
                        )
                    av_ps, rb_ps = acc[h]
                    r0 = max(0, j - 4 * g)
                    off = r0 * P
                    w = CW - off
                    st = sts.pop((h, j))
                    ptile = pt_pool.tile([P, CW], BF16, tag="pt",
                                         name=f"pt_{g}_{h}_{j}")
                    nc.scalar.activation(
                        ptile[:, 0:w], st[:, 0:w],
                        mybir.ActivationFunctionType.Exp, scale=SCALE,
                    )
                    if j >= 4 * g:
                        # zero out the masked (strictly-upper) part of the
                        # diagonal block: cheaper than -inf add pre-exp
                        nc.vector.tensor_tensor(
                            ptile[:, 0:P], ptile[:, 0:P], maskb_sb[:],
                            mybir.AluOpType.mult,
                        )
                    nc.tensor.matmul(
                        rb_ps[:, off:off + w], onessq_sb[:], ptile[:, 0:w],
                        start=(j == 0), stop=(j == njt - 1),
                    )
                    nc.tensor.matmul(
                        av_ps[:, off:off + w], v_sb[:, j, :], ptile[:, 0:w],
                        start=(j == 0), stop=(j == njt - 1),
                    )
                    if j == njt - 1:
                        # normalize: approx reciprocal of broadcast rowsums
                        recipb = wk_pool.tile([P, CW], F32, tag="wk",
                                              name=f"rc_{g}_{h}")
                        nc.vector.reciprocal_approx_fast(recipb[:], rb_ps[:])
                        if attnT[g] is None:
                            attnT[g] = qa_pool.tile([P, NH, CW], BF16,
                                                    tag="qa",
                                                    name=f"attnT_{g}")
                        nc.vector.tensor_tensor(
                            attnT[g][:, h, :], av_ps[:], recipb[:],
                            mybir.AluOpType.mult,
                        )
                        del acc[h]

            def wo_tile(i):
                g, r = i // NCH, i % NCH
                o_stage = os_pool.tile([P, H], BF16, tag="os", name=f"ost_{i}")
                for jc in range(NJC):
                    o_ps = ps_pool.tile([P, CW], F32, tag="mm",
                                        name=f"op_{i}_{jc}")
                    for f in range(NH):
                        nc.tensor.matmul(
                            o_ps[:],
                            attnT[g][:, f, r * P:(r + 1) * P],
                            wot_sb[:, f * H + jc * CW:f * H + (jc + 1) * CW],
                            start=(f == 0), stop=(f == NH - 1),
                        )
                    eng = nc.scalar.copy if jc % 2 == 0 else nc.vector.tensor_copy
                    eng(o_stage[:, jc * CW:(jc + 1) * CW], o_ps[:])
                    # per-jc DMA: starts draining while later jc still compute
                    nc.sync.dma_start(
                        out.ap()[i * P:(i + 1) * P, jc * CW:(jc + 1) * CW],
                        o_stage[:, jc * CW:(jc + 1) * CW])

            # ====== main schedule: proj s-tiles with attention + output
            # projection spread finely across the next chunk's tiles; the
            # attn/wo hooks sit between a tile's matmuls and its stats/rope
            # so the exp stream gets the ACT engine first ======
            warm23 = {}
            for i in range(NS):
                proj_mm(i)
                if i >= NCH:
                    g, ph = (i - NCH) // NCH, i % NCH
                    if ph == 0:
                        attn_phase(g, (0, 1))
                        # pre-issue the next phase's first S matmuls so it
                        # starts with its exp pipeline already primed
                        warm23 = {}
                        s_mm(g, 2, 0, warm23)
                        s_mm(g, 2, 1, warm23)
                        proj_post(i)
                    elif ph == 1:
                        attn_phase(g, (2, 3), warm=warm23)
                        proj_post(i)
                    elif ph == 2:
                        # rope before the wo tiles: its DVE chain must be
                        # done by the next tile's transpose flush
                        proj_post(i)
                        wo_tile(NCH * g)
                        wo_tile(NCH * g + 1)
                    else:
                        proj_post(i)
                        wo_tile(NCH * g + 2)
                        wo_tile(NCH * g + 3)
                else:
                    proj_post(i)
            flush_tp()
            attn_phase(NCH - 1, (0, 1, 2, 3))
            for r in range(NCH):
                wo_tile((NCH - 1) * NCH + r)
    nc.compile()
    return nc


def kernel(hidden_states, cos, sin, Wq, Wk, Wv, Wo, q_norm_w, k_norm_w):
    hs = np.asarray(hidden_states, dtype=np.float32)[0]      # [S, H]
    cos0 = np.asarray(cos, dtype=np.float32)[0]              # [S, 128]
    sin0 = np.asarray(sin, dtype=np.float32)[0]
    Wq = np.asarray(Wq, dtype=np.float32)
    Wk = np.asarray(Wk, dtype=np.float32)
    Wv = np.asarray(Wv, dtype=np.float32)
    Wo = np.asarray(Wo, dtype=np.float32)
    qw = np.asarray(q_norm_w, dtype=np.float32)
    kw = np.asarray(k_norm_w, dtype=np.float32)

    BF = ml_dtypes.bfloat16

    # slab[i][p][t*128+s] = hs[i*128+s, t*128+p]
    hst_t = np.ascontiguousarray(
        hs.reshape(NS, P, NHT, P).transpose(0, 3, 2, 1).reshape(NS, P, NHT * P)
    ).astype(BF)
    sgn = np.concatenate([-np.ones(64, np.float32), np.ones(64, np.float32)])

    def tables(w):
        wr = np.concatenate([w[64:], w[:64]])                # w[(i+64)%128]
        return cos0 * w[None, :], sin0 * (sgn * wr)[None, :]

    cosq_t, sinq_t = tables(qw)
    cosk_t, sink_t = tables(kw)
    tab_t = np.ascontiguousarray(
        np.concatenate([cosq_t, sinq_t, cosk_t, sink_t], axis=1)
        .astype(np.float32).reshape(NS, P, 4 * P)
    ).astype(BF)
    idx = np.arange(P)
    maskb_np = (idx[None, :] >= idx[:, None]).astype(BF)
    ident_np = np.eye(P, dtype=np.float32).astype(BF)
    onessq_np = np.ones((P, P), np.float32).astype(BF)

    if "nc" not in _CACHE:
        _CACHE["nc"] = _build()
    nc = _CACHE["nc"]

    in_maps = []
    for c in range(8):
        wq_c = Wq[c * DQ:(c + 1) * DQ, :]                    # [512, H]
        wqt_t = np.ascontiguousarray(
            wq_c.reshape(DQ, NHT, P).transpose(2, 1, 0).reshape(P, NHT * DQ)
        ).astype(BF)
        kv_c = np.concatenate([Wk[c * P:(c + 1) * P, :], Wv[c * P:(c + 1) * P, :]], axis=0)
        wkvt_t = np.ascontiguousarray(
            kv_c.reshape(DKV, NHT, P).transpose(2, 1, 0).reshape(P, NHT * DKV)
        ).astype(BF)
        wot_c = np.ascontiguousarray(Wo[:, c * DQ:(c + 1) * DQ].T)  # [512, H]
        wot_t = np.ascontiguousarray(
            wot_c.reshape(NH, P, H).transpose(1, 0, 2).reshape(P, NH * H)
        ).astype(BF)
        in_maps.append(dict(
            hst=hst_t, wqt=wqt_t, wkvt=wkvt_t, wot=wot_t, tab=tab_t,
            maskb=maskb_np, ident=ident_np, onessq=onessq_np,
        ))

    try:
        r = run_bass_kernel_spmd(nc, in_maps, core_ids=list(range(8)))
    except Exception:
        r = run_bass_kernel_spmd(nc, in_maps, core_ids=list(range(8)))
    acc = np.zeros((S, H), dtype=np.float32)
    for c in range(8):
        acc += np.asarray(r.results[c]["out"], dtype=np.float32)
    return acc[None, :, :]


# revision 39
# speedup vs baseline: 1.0082x; 1.0082x over previous
"""GQA causal-attention prefill kernel for 8 TRN2 NeuronCores.

Sharding (zero cross-core comm): 8 KV heads -> 1 per core, each with its 4
GQA query heads. Per core: Wq slice [512,2560], Wk/Wv slice [128,2560], Wo
column-slice [2560,512]. Each core computes a full [2048,2560] partial of
the output projection; the host sums the 8 partials.

Per-core math (all matmuls in bf16, f32 PSUM accumulation):
  proj:  q/k/v seq-major via stationary=hsT tiles, moving=W^T slabs
  norm+rope on seq-major tiles (DVE), fold norm weights into cos/sin tables
  rstd via exp(-0.5*ln(var+eps)) so ACT stays in one table (exp/ln/square)
  PE-transpose roped q,k -> feature-major qT,kT (bf16 PSUM)
  S^T = kT_tile.T @ qT  (lower-triangle blocks only, lookahead-2 issue)
  P^T = exp(scale*S^T) (ACT); diagonal blocks zeroed by binary-mask multiply
  rowsum via ones matmul; AV: attnT = v.T @ P^T; normalize via approx recip
  out_partial = attnT.T @ WoT, interleaved per chunk so output DMA spreads
  across the whole kernel instead of crunching at the tail.

DMAs are issued in need-order (first weight chunks + first hsT slab first)
so the first projection matmul starts ~2us in. Evictions run on the
otherwise-idle GpSimd engine.
"""

import ml_dtypes
import numpy as np

import concourse.bass as bass
import concourse.mybir as mybir
import concourse.tile as tile
from concourse import bacc
from concourse.bass_utils import run_bass_kernel_spmd

P = 128
S = 2048
H = 2560
NS = S // P          # 16 s-tiles
NHT = H // P         # 20 hidden tiles
NH = 4               # q heads per core
DQ = NH * P          # 512
DKV = 2 * P          # 256 (k|v)
NCH = 4              # sq chunks of 512
CW = 512
NJC = H // CW        # 5 output col chunks
SCALE = float(P) ** -0.5
EPS = 1e-6

F32 = mybir.dt.float32
BF16 = mybir.dt.bfloat16

# rsqrt(var) ~ deg-3 poly on var in [0.6, 2.9] + 1 Newton step (max rel err
# ~1e-3); keeps rstd off the ACT engine so ACT never leaves its exp table.
RSQ_C3 = -0.06104849
RSQ_C2 = 0.44709427
RSQ_C1 = -1.21807019
RSQ_C0 = 1.84091155

_CACHE = {}


def _build():
    nc = bacc.Bacc("TRN2", target_bir_lowering=False)

    hst = nc.declare_dram_parameter("hst", [NS, P, NHT * P], BF16, isOutput=False)
    wqt = nc.declare_dram_parameter("wqt", [P, NHT * DQ], BF16, isOutput=False)
    wkvt = nc.declare_dram_parameter("wkvt", [P, NHT * DKV], BF16, isOutput=False)
    wot = nc.declare_dram_parameter("wot", [P, NH * H], BF16, isOutput=False)
    tab = nc.declare_dram_parameter("tab", [NS, P, 4 * P], BF16, isOutput=False)
    maskb = nc.declare_dram_parameter("maskb", [P, P], BF16, isOutput=False)
    ident = nc.declare_dram_parameter("ident", [P, P], BF16, isOutput=False)
    onessq = nc.declare_dram_parameter("onessq", [P, P], BF16, isOutput=False)
    out = nc.declare_dram_parameter("out", [S, H], BF16, isOutput=True)

    with tile.TileContext(nc) as tc:
        with (
            tc.tile_pool(name="wq", bufs=1) as wq_pool,        # WqT slab
            tc.tile_pool(name="wkv", bufs=1) as wkv_pool,      # WkvT slab
            tc.tile_pool(name="wow", bufs=1) as wow_pool,      # WoT slab
            tc.tile_pool(name="qa", bufs=5) as qa_pool,        # qT_g / attnT_g
            tc.tile_pool(name="kv", bufs=1) as kv_pool,        # kT + v
            tc.tile_pool(name="big", bufs=6) as big_pool,      # hsT slabs
            tc.tile_pool(name="os", bufs=3) as os_pool,        # out staging
            tc.tile_pool(name="tab", bufs=4) as tab_pool,      # table blocks
            tc.tile_pool(name="wk", bufs=9) as wk_pool,        # rope scratch
            tc.tile_pool(name="qw", bufs=4) as qw_pool,        # roped q/k
            tc.tile_pool(name="pt", bufs=10) as pt_pool,       # P^T tiles
            tc.tile_pool(name="sm", bufs=8) as sm_pool,        # small stats
            tc.tile_pool(name="cst", bufs=1) as cst_pool,      # consts
            tc.tile_pool(name="ps", bufs=7, space="PSUM") as ps_pool,
            tc.tile_pool(name="tp", bufs=1, space="PSUM") as tp_pool,
        ):
            slabs, tabts = {}, {}

            def load_inputs(i, split=4):
                if i in slabs or i >= NS:
                    return
                # split across DMA queues: a single queue only gets ~1/16
                # of HBM bandwidth when all queues are busy
                slabs[i] = big_pool.tile([P, NHT * P], BF16, tag="big",
                                         name=f"slab_{i}")
                qn = NHT * P // split
                for q4 in range(split):
                    nc.sync.dma_start(
                        slabs[i][:, q4 * qn:(q4 + 1) * qn],
                        hst.ap()[i][:, q4 * qn:(q4 + 1) * qn],
                    )
                tabts[i] = tab_pool.tile([P, 4 * P], BF16, tag="tab",
                                         name=f"tabt_{i}")
                nc.sync.dma_start(tabts[i][:], tab.ap()[i])

            wqt_sb = wq_pool.tile([P, NHT * DQ], BF16, tag="w")
            wkvt_sb = wkv_pool.tile([P, NHT * DKV], BF16, tag="w")
            wot_sb = wow_pool.tile([P, NH * H], BF16, tag="w")

            # ---- DMAs in need-order: first wq chunks + slab0 pieces, then
            # the rest of the weights, early slabs, and WoT ----
            slabs[0] = big_pool.tile([P, NHT * P], BF16, tag="big",
                                     name="slab_0")
            # absolute minimum for the first (kv) matmul: WkvT t=0 + slab0
            # t=0; the small WkvT streams in fully while WqT follows
            hkv = NHT * DKV // 4
            nc.sync.dma_start(wkvt_sb[:, 0:hkv], wkvt.ap()[:, 0:hkv])
            nc.sync.dma_start(slabs[0][:, 0:P], hst.ap()[0][:, 0:P])
            nc.sync.dma_start(slabs[0][:, P:2 * P], hst.ap()[0][:, P:2 * P])
            for c4 in range(1, 4):
                nc.sync.dma_start(wkvt_sb[:, c4 * hkv:(c4 + 1) * hkv],
                                  wkvt.ap()[:, c4 * hkv:(c4 + 1) * hkv])
            CC = 2 * DQ  # two t-chunks of WqT per DMA
            sq = NHT * P // 4
            nc.sync.dma_start(slabs[0][:, 2 * P:sq], hst.ap()[0][:, 2 * P:sq])
            for cc in range(10):
                nc.sync.dma_start(wqt_sb[:, cc * CC:(cc + 1) * CC],
                                  wqt.ap()[:, cc * CC:(cc + 1) * CC])
                if cc < 3:
                    nc.sync.dma_start(
                        slabs[0][:, (cc + 1) * sq:(cc + 2) * sq],
                        hst.ap()[0][:, (cc + 1) * sq:(cc + 2) * sq])
                elif cc == 3:
                    tabts[0] = tab_pool.tile([P, 4 * P], BF16, tag="tab",
                                             name="tabt_0")
                    nc.sync.dma_start(tabts[0][:], tab.ap()[0])
                elif cc == 4:
                    # consts (needed from the first transpose/attn on)
                    maskb_sb = cst_pool.tile([P, P], BF16, tag="maskb")
                    nc.sync.dma_start(maskb_sb[:], maskb.ap())
                    ident_sb = cst_pool.tile([P, P], BF16, tag="ident")
                    nc.sync.dma_start(ident_sb[:], ident.ap())
                    onessq_sb = cst_pool.tile([P, P], BF16, tag="onessq")
                    nc.sync.dma_start(onessq_sb[:], onessq.ap())
                elif cc == 5:
                    load_inputs(1)
            load_inputs(2)
            load_inputs(3)
            hwo = NH * H // 4
            for cc in range(4):
                nc.sync.dma_start(wot_sb[:, cc * hwo:(cc + 1) * hwo],
                                  wot.ap()[:, cc * hwo:(cc + 1) * hwo])

            # persistent attention operands
            kT = kv_pool.tile([P, S], BF16, tag="kt")          # [d, t]
            v_sb = kv_pool.tile([P, NS, P], BF16, tag="v")     # [t, tile, d]

            qT = [None] * NCH
            attnT = [None] * NCH
            pending_tp = [None]  # deferred transpose of the previous tile

            def flush_tp():
                if pending_tp[0] is None:
                    return
                i, g, r, q_ro, k_ro = pending_tp[0]
                pending_tp[0] = None
                tp = tp_pool.tile([P, 5 * P], BF16, tag="tp", name=f"tp_{i}")
                for h in range(NH):
                    nc.tensor.transpose(
                        tp[:, h * P:(h + 1) * P], q_ro[:, h * P:(h + 1) * P],
                        ident_sb[:],
                    )
                nc.tensor.transpose(tp[:, DQ:DQ + P], k_ro[:], ident_sb[:])
                if qT[g] is None:
                    qT[g] = qa_pool.tile([P, NCH, NH, P], BF16, tag="qa",
                                         name=f"qT_{g}")
                nc.vector.tensor_copy(
                    qT[g][:, r, :, :].rearrange("p h d -> p (h d)"),
                    tp[:, 0:DQ],
                )
                nc.vector.tensor_copy(kT[:, i * P:(i + 1) * P], tp[:, DQ:DQ + P])

            proj_ps = {}

            def proj_mm(i):
                load_inputs(i + 2)
                load_inputs(i + 3)
                slab = slabs.pop(i)

                q_ps = ps_pool.tile([P, DQ], F32, tag="mm")
                kv_ps = ps_pool.tile([P, DKV], F32, tag="mm")
                # kv first for the DMA-starved opening tiles (WkvT is small
                # and lands before the WqT stream completes)
                passes = [(kv_ps, wkvt_sb, DKV), (q_ps, wqt_sb, DQ)]
                if i >= 3:
                    passes.reverse()
                for pi, (ps, wsb, dw) in enumerate(passes):
                    for t in range(NHT):
                        nc.tensor.matmul(
                            ps[:], slab[:, t * P:(t + 1) * P],
                            wsb[:, t * dw:(t + 1) * dw],
                            start=(t == 0), stop=(t == NHT - 1),
                        )
                    if pi == 0:
                        # the previous tile's transposes land between the
                        # two projection passes: its rope chain (DVE) hides
                        # under pass 1, and the qT/kT copies (DVE) hide
                        # under pass 2 — so a directly-following attention
                        # phase finds qT ready
                        flush_tp()
                proj_ps[i] = (q_ps, kv_ps)

            def proj_post(i):
                g, r = i // NCH, i % NCH
                q_ps, kv_ps = proj_ps.pop(i)
                tabt = tabts.pop(i)
                cq_t, sq_t = tabt[:, 0:P], tabt[:, P:2 * P]
                ck_t, sk_t = tabt[:, 2 * P:3 * P], tabt[:, 3 * P:4 * P]

                # v evict (f32 psum -> bf16); gpsimd can't read PSUM
                nc.scalar.activation(v_sb[:, i, :], kv_ps[:, P:DKV],
                                     mybir.ActivationFunctionType.Copy)

                # ---- rms-norm stats (ACT squares, DVE rsqrt poly) ----
                q2 = wk_pool.tile([P, DQ], F32, tag="wk", name=f"q2_{i}")
                nc.scalar.activation(
                    q2[:], q_ps[:, 0:DQ],
                    mybir.ActivationFunctionType.Square,
                )
                ss = sm_pool.tile([P, NH + 1], F32, tag="ssq")
                nc.vector.tensor_reduce(
                    ss[:, 0:NH], q2[:].rearrange("p (h d) -> p h d", h=NH),
                    mybir.AxisListType.X, mybir.AluOpType.add,
                )
                junk = sm_pool.tile([P, P], F32, tag="junk")
                nc.scalar.activation(
                    junk[:], kv_ps[:, 0:P],
                    mybir.ActivationFunctionType.Square,
                    accum_out=ss[:, NH:NH + 1],
                )
                # rstd = rsqrt(ss/P) via poly+Newton on DVE (keeps ACT in
                # its exp table; Sqrt/Ln would force 1.3us table reloads).
                # Poly in raw ss: coefficients pre-divided by powers of P.
                NW = NH + 1
                h1 = sm_pool.tile([P, NW], F32, tag="h1")
                nc.vector.tensor_scalar(
                    h1[:], ss[:], RSQ_C3 / P ** 3, RSQ_C2 / P ** 2,
                    mybir.AluOpType.mult, mybir.AluOpType.add,
                )
                nc.vector.tensor_tensor(h1[:], h1[:], ss[:],
                                        mybir.AluOpType.mult)
                nc.vector.tensor_scalar_add(h1[:], h1[:], RSQ_C1 / P)
                y0 = sm_pool.tile([P, NW], F32, tag="y0")
                nc.vector.tensor_tensor(y0[:], h1[:], ss[:],
                                        mybir.AluOpType.mult)
                nc.vector.tensor_scalar_add(y0[:], y0[:], RSQ_C0)
                # Newton: rstd = y0 * (1.5 - (ss/(2P))*y0^2)
                t1 = sm_pool.tile([P, NW], F32, tag="t1")
                nc.vector.tensor_tensor(t1[:], y0[:], y0[:],
                                        mybir.AluOpType.mult)
                nc.vector.tensor_tensor(t1[:], t1[:], ss[:],
                                        mybir.AluOpType.mult)
                nc.vector.tensor_scalar(
                    t1[:], t1[:], -0.5 / P, 1.5,
                    mybir.AluOpType.mult, mybir.AluOpType.add,
                )
                rstd = sm_pool.tile([P, NW], F32, tag="rsq")
                nc.vector.tensor_tensor(rstd[:], y0[:], t1[:],
                                        mybir.AluOpType.mult)
                rstd_q, rstd_k = rstd[:, 0:NH], rstd[:, NH:NH + 1]

                # ---- fused norm-scale + rope (DVE, bf16 after first mult) ----
                def rope(ps_slice, nh, rstd, cos_t, sin_t, nm):
                    w = nh * P
                    qn = wk_pool.tile([P, w], BF16, tag="wk", name=f"qn_{nm}_{i}")
                    q3 = qn[:].rearrange("p (h d) -> p h d", h=nh)
                    nc.vector.tensor_tensor(
                        q3, ps_slice.rearrange("p (h d) -> p h d", h=nh),
                        rstd[:, :, None].broadcast_to([P, nh, P]),
                        mybir.AluOpType.mult,
                    )
                    r1 = wk_pool.tile([P, w], BF16, tag="wk", name=f"r1_{nm}_{i}")
                    nc.vector.tensor_tensor(
                        r1[:].rearrange("p (h d) -> p h d", h=nh), q3,
                        cos_t[:, None, :].broadcast_to([P, nh, P]),
                        mybir.AluOpType.mult,
                    )
                    r2 = wk_pool.tile([P, w], BF16, tag="wk", name=f"r2_{nm}_{i}")
                    r23 = r2[:].rearrange("p (h d) -> p h d", h=nh)
                    nc.vector.tensor_tensor(
                        r23[:, :, 0:64], q3[:, :, 64:P],
                        sin_t[:, None, 0:64].broadcast_to([P, nh, 64]),
                        mybir.AluOpType.mult,
                    )
                    nc.vector.tensor_tensor(
                        r23[:, :, 64:P], q3[:, :, 0:64],
                        sin_t[:, None, 64:P].broadcast_to([P, nh, 64]),
                        mybir.AluOpType.mult,
                    )
                    ro = qw_pool.tile([P, w], BF16, tag="qw", name=f"ro_{nm}_{i}")
                    nc.vector.tensor_tensor(
                        ro[:], r1[:], r2[:], mybir.AluOpType.add,
                    )
                    return ro

                q_ro = rope(q_ps[:, 0:DQ], NH, rstd_q, cq_t, sq_t, "q")
                k_ro = rope(kv_ps[:, 0:P], 1, rstd_k, ck_t, sk_t, "k")
                # transposes deferred to the next tile's matmul shadow
                pending_tp[0] = (i, g, r, q_ro, k_ro)

            def s_mm(g, h, j, sts):
                r0 = max(0, j - 4 * g)
                w = CW - r0 * P
                st = ps_pool.tile([P, CW], F32, tag="mm",
                                  name=f"st_{g}_{h}_{j}")
                nc.tensor.matmul(
                    st[:, 0:w],
                    kT[:, j * P:(j + 1) * P],
                    qT[g][:, r0:NCH, h, :],
                )
                sts[(h, j)] = st

            def attn_phase(g, hs, warm=None, look=2):
                """Attention units (heads hs) as one flat S-matmul stream
                with cross-unit lookahead, so the PE never drains at the
                unit boundary waiting for the first exp. `warm` carries S
                tiles pre-issued by the previous phase."""
                njt = 4 * g + 4  # t-tiles 0..4g+3
                seq = [(h, j) for h in hs for j in range(njt)]
                sts = warm if warm is not None else {}
                acc = {}  # h -> (av_ps, rb_ps)

                LOOK = look
                for k in range(min(LOOK, len(seq))):
                    if seq[k] not in sts:
                        s_mm(g, seq[k][0], seq[k][1], sts)
                for k, (h, j) in enumerate(seq):
                    if k + LOOK < len(seq) and seq[k + LOOK] not in sts:
                        h2, j2 = seq[k + LOOK]
                        s_mm(g, h2, j2, sts)
                    if j == 0:
                        acc[h] = (
                            ps_pool.tile([P, CW], F32, tag="mm",
                                         name=f"av_{g}_{h}"),
                            ps_pool.tile([P, CW], F32, tag="mm",
                                         name=f"rb_{g}_{h}"),
                        )
                    av_ps, rb_ps = acc[h]
                    r0 = max(0, j - 4 * g)
                    off = r0 * P
                    w = CW - off
                    st = sts.pop((h, j))
                    ptile = pt_pool.tile([P, CW], BF16, tag="pt",
                                         name=f"pt_{g}_{h}_{j}")
                    nc.scalar.activation(
                        ptile[:, 0:w], st[:, 0:w],
                        mybir.ActivationFunctionType.Exp, scale=SCALE,
                    )
                    if j >= 4 * g:
                        # zero out the masked (strictly-upper) part of the
                        # diagonal block: cheaper than -inf add pre-exp
                        nc.vector.tensor_tensor(
                            ptile[:, 0:P], ptile[:, 0:P], maskb_sb[:],
                            mybir.AluOpType.mult,
                        )
                    nc.tensor.matmul(
                        rb_ps[:, off:off + w], onessq_sb[:], ptile[:, 0:w],
                        start=(j == 0), stop=(j == njt - 1),
                    )
                    nc.tensor.matmul(
                        av_ps[:, off:off + w], v_sb[:, j, :], ptile[:, 0:w],
                        start=(j == 0), stop=(j == njt - 1),
                    )
                    if j == njt - 1:
                        # normalize: approx reciprocal of broadcast rowsums
                        recipb = wk_pool.tile([P, CW], F32, tag="wk",
                                              name=f"rc_{g}_{h}")
                        nc.vector.reciprocal_approx_fast(recipb[:], rb_ps[:])
                        if attnT[g] is None:
                            attnT[g] = qa_pool.tile([P, NH, CW], BF16,
                                                    tag="qa",
                                                    name=f"attnT_{g}")
                        nc.vector.tensor_tensor(
                            attnT[g][:, h, :], av_ps[:], recipb[:],
                            mybir.AluOpType.mult,
                        )
                        del acc[h]

            def wo_tile(i, tail=False):
                g, r = i // NCH, i % NCH
                o_stage = os_pool.tile([P, H], BF16, tag="os", name=f"ost_{i}")
                for jc in range(NJC):
                    o_ps = ps_pool.tile([P, CW], F32, tag="mm",
                                        name=f"op_{i}_{jc}")
                    for f in range(NH):
                        nc.tensor.matmul(
                            o_ps[:],
                            attnT[g][:, f, r * P:(r + 1) * P],
                            wot_sb[:, f * H + jc * CW:f * H + (jc + 1) * CW],
                            start=(f == 0), stop=(f == NH - 1),
                        )
                    # in the tail the ACT engine is co-critical with the last
                    # attention chunk's exps: keep all evicts on DVE there
                    eng = (nc.vector.tensor_copy if tail or jc % 2 == 1
                           else nc.scalar.copy)
                    eng(o_stage[:, jc * CW:(jc + 1) * CW], o_ps[:])
                    # per-jc DMA: starts draining while later jc still compute
                    nc.sync.dma_start(
                        out.ap()[i * P:(i + 1) * P, jc * CW:(jc + 1) * CW],
                        o_stage[:, jc * CW:(jc + 1) * CW])

            # ====== main schedule: proj s-tiles with attention + output
            # projection spread finely across the next chunk's tiles; the
            # attn/wo hooks sit between a tile's matmuls and its stats/rope
            # so the exp stream gets the ACT engine first ======
            warm23 = {}
            for i in range(NS):
                proj_mm(i)
                if i >= NCH:
                    g, ph = (i - NCH) // NCH, i % NCH
                    if ph == 0:
                        attn_phase(g, (0, 1))
                        # pre-issue the next phase's first S matmuls so it
                        # starts with its exp pipeline already primed
                        warm23 = {}
                        s_mm(g, 2, 0, warm23)
                        s_mm(g, 2, 1, warm23)
                        proj_post(i)
                    elif ph == 1:
                        attn_phase(g, (2, 3), warm=warm23)
                        proj_post(i)
                    elif ph == 2:
                        # rope before the wo tiles: its DVE chain must be
                        # done by the next tile's transpose flush
                        proj_post(i)
                        wo_tile(NCH * g)
                        wo_tile(NCH * g + 1)
                    else:
                        proj_post(i)
                        wo_tile(NCH * g + 2)
                        wo_tile(NCH * g + 3)
                else:
                    proj_post(i)
            flush_tp()
            attn_phase(NCH - 1, (0, 1, 2, 3), look=3)
            for r in range(NCH):
                wo_tile((NCH - 1) * NCH + r, tail=True)
    nc.compile()
    return nc


def kernel(hidden_states, cos, sin, Wq, Wk, Wv, Wo, q_norm_w, k_norm_w):
    hs = np.asarray(hidden_states, dtype=np.float32)[0]      # [S, H]
    cos0 = np.asarray(cos, dtype=np.float32)[0]              # [S, 128]
    sin0 = np.asarray(sin, dtype=np.float32)[0]
    Wq = np.asarray(Wq, dtype=np.float32)
    Wk = np.asarray(Wk, dtype=np.float32)
    Wv = np.asarray(Wv, dtype=np.float32)
    Wo = np.asarray(Wo, dtype=np.float32)
    qw = np.asarray(q_norm_w, dtype=np.float32)
    kw = np.asarray(k_norm_w, dtype=np.float32)

    BF = ml_dtypes.bfloat16

    # slab[i][p][t*128+s] = hs[i*128+s, t*128+p]
    hst_t = np.ascontiguousarray(
        hs.reshape(NS, P, NHT, P).transpose(0, 3, 2, 1).reshape(NS, P, NHT * P)
    ).astype(BF)
    sgn = np.concatenate([-np.ones(64, np.float32), np.ones(64, np.float32)])

    def tables(w):
        wr = np.concatenate([w[64:], w[:64]])                # w[(i+64)%128]
        return cos0 * w[None, :], sin0 * (sgn * wr)[None, :]

    cosq_t, sinq_t = tables(qw)
    cosk_t, sink_t = tables(kw)
    tab_t = np.ascontiguousarray(
        np.concatenate([cosq_t, sinq_t, cosk_t, sink_t], axis=1)
        .astype(np.float32).reshape(NS, P, 4 * P)
    ).astype(BF)
    idx = np.arange(P)
    maskb_np = (idx[None, :] >= idx[:, None]).astype(BF)
    ident_np = np.eye(P, dtype=np.float32).astype(BF)
    onessq_np = np.ones((P, P), np.float32).astype(BF)

    if "nc" not in _CACHE:
        _CACHE["nc"] = _build()
    nc = _CACHE["nc"]

    in_maps = []
    for c in range(8):
        wq_c = Wq[c * DQ:(c + 1) * DQ, :]                    # [512, H]
        wqt_t = np.ascontiguousarray(
            wq_c.reshape(DQ, NHT, P).transpose(2, 1, 0).reshape(P, NHT * DQ)
        ).astype(BF)
        kv_c = np.concatenate([Wk[c * P:(c + 1) * P, :], Wv[c * P:(c + 1) * P, :]], axis=0)
        wkvt_t = np.ascontiguousarray(
            kv_c.reshape(DKV, NHT, P).transpose(2, 1, 0).reshape(P, NHT * DKV)
        ).astype(BF)
        wot_c = np.ascontiguousarray(Wo[:, c * DQ:(c + 1) * DQ].T)  # [512, H]
        wot_t = np.ascontiguousarray(
            wot_c.reshape(NH, P, H).transpose(1, 0, 2).reshape(P, NH * H)
        ).astype(BF)
        in_maps.append(dict(
            hst=hst_t, wqt=wqt_t, wkvt=wkvt_t, wot=wot_t, tab=tab_t,
            maskb=maskb_np, ident=ident_np, onessq=onessq_np,
        ))

    try:
        r = run_bass_kernel_spmd(nc, in_maps, core_ids=list(range(8)))
    except Exception:
        r = run_bass_kernel_spmd(nc, in_maps, core_ids=list(range(8)))
    acc = np.zeros((S, H), dtype=np.float32)
    for c in range(8):
        acc += np.asarray(r.results[c]["out"], dtype=np.float32)
    return acc[None, :, :]


# revision 42
# speedup vs baseline: 1.0094x; 1.0011x over previous
"""GQA causal-attention prefill kernel for 8 TRN2 NeuronCores.

Sharding (zero cross-core comm): 8 KV heads -> 1 per core, each with its 4
GQA query heads. Per core: Wq slice [512,2560], Wk/Wv slice [128,2560], Wo
column-slice [2560,512]. Each core computes a full [2048,2560] partial of
the output projection; the host sums the 8 partials.

Per-core math (all matmuls in bf16, f32 PSUM accumulation):
  proj:  q/k/v seq-major via stationary=hsT tiles, moving=W^T slabs
  norm+rope on seq-major tiles (DVE), fold norm weights into cos/sin tables
  rstd via exp(-0.5*ln(var+eps)) so ACT stays in one table (exp/ln/square)
  PE-transpose roped q,k -> feature-major qT,kT (bf16 PSUM)
  S^T = kT_tile.T @ qT  (lower-triangle blocks only, lookahead-2 issue)
  P^T = exp(scale*S^T) (ACT); diagonal blocks zeroed by binary-mask multiply
  rowsum via ones matmul; AV: attnT = v.T @ P^T; normalize via approx recip
  out_partial = attnT.T @ WoT, interleaved per chunk so output DMA spreads
  across the whole kernel instead of crunching at the tail.

DMAs are issued in need-order (first weight chunks + first hsT slab first)
so the first projection matmul starts ~2us in. Evictions run on the
otherwise-idle GpSimd engine.
"""

import ml_dtypes
import numpy as np

import concourse.bass as bass
import concourse.mybir as mybir
import concourse.tile as tile
from concourse import bacc
from concourse.bass_utils import run_bass_kernel_spmd

P = 128
S = 2048
H = 2560
NS = S // P          # 16 s-tiles
NHT = H // P         # 20 hidden tiles
NH = 4               # q heads per core
DQ = NH * P          # 512
DKV = 2 * P          # 256 (k|v)
NCH = 4              # sq chunks of 512
CW = 512
NJC = H // CW        # 5 output col chunks
SCALE = float(P) ** -0.5
EPS = 1e-6

F32 = mybir.dt.float32
BF16 = mybir.dt.bfloat16

# rsqrt(var) ~ deg-3 poly on var in [0.6, 2.9] + 1 Newton step (max rel err
# ~1e-3); keeps rstd off the ACT engine so ACT never leaves its exp table.
RSQ_C3 = -0.06104849
RSQ_C2 = 0.44709427
RSQ_C1 = -1.21807019
RSQ_C0 = 1.84091155

_CACHE = {}


def _build():
    nc = bacc.Bacc("TRN2", target_bir_lowering=False)

    hst = nc.declare_dram_parameter("hst", [NS, P, NHT * P], BF16, isOutput=False)
    wqt = nc.declare_dram_parameter("wqt", [P, NHT * DQ], BF16, isOutput=False)
    wkvt = nc.declare_dram_parameter("wkvt", [P, NHT * DKV], BF16, isOutput=False)
    wot = nc.declare_dram_parameter("wot", [P, NH * H], BF16, isOutput=False)
    tab = nc.declare_dram_parameter("tab", [NS, P, 4 * P], BF16, isOutput=False)
    maskb = nc.declare_dram_parameter("maskb", [P, P], BF16, isOutput=False)
    ident = nc.declare_dram_parameter("ident", [P, P], BF16, isOutput=False)
    onessq = nc.declare_dram_parameter("onessq", [P, P], BF16, isOutput=False)
    out = nc.declare_dram_parameter("out", [S, H], BF16, isOutput=True)

    with tile.TileContext(nc) as tc:
        with (
            tc.tile_pool(name="wq", bufs=1) as wq_pool,        # WqT slab
            tc.tile_pool(name="wkv", bufs=1) as wkv_pool,      # WkvT slab
            tc.tile_pool(name="wow", bufs=1) as wow_pool,      # WoT slab
            tc.tile_pool(name="qa", bufs=5) as qa_pool,        # qT_g / attnT_g
            tc.tile_pool(name="kv", bufs=1) as kv_pool,        # kT + v
            tc.tile_pool(name="big", bufs=6) as big_pool,      # hsT slabs
            tc.tile_pool(name="os", bufs=3) as os_pool,        # out staging
            tc.tile_pool(name="tab", bufs=4) as tab_pool,      # table blocks
            tc.tile_pool(name="wk", bufs=9) as wk_pool,        # rope scratch
            tc.tile_pool(name="qw", bufs=4) as qw_pool,        # roped q/k
            tc.tile_pool(name="pt", bufs=10) as pt_pool,       # P^T tiles
            tc.tile_pool(name="sm", bufs=8) as sm_pool,        # small stats
            tc.tile_pool(name="cst", bufs=1) as cst_pool,      # consts
            tc.tile_pool(name="ps", bufs=7, space="PSUM") as ps_pool,
            tc.tile_pool(name="tp", bufs=1, space="PSUM") as tp_pool,
        ):
            slabs, tabts = {}, {}

            def load_inputs(i, split=4):
                if i in slabs or i >= NS:
                    return
                # split across DMA queues: a single queue only gets ~1/16
                # of HBM bandwidth when all queues are busy
                slabs[i] = big_pool.tile([P, NHT * P], BF16, tag="big",
                                         name=f"slab_{i}")
                qn = NHT * P // split
                for q4 in range(split):
                    nc.sync.dma_start(
                        slabs[i][:, q4 * qn:(q4 + 1) * qn],
                        hst.ap()[i][:, q4 * qn:(q4 + 1) * qn],
                    )
                tabts[i] = tab_pool.tile([P, 4 * P], BF16, tag="tab",
                                         name=f"tabt_{i}")
                nc.sync.dma_start(tabts[i][:], tab.ap()[i])

            wqt_sb = wq_pool.tile([P, NHT * DQ], BF16, tag="w")
            wkvt_sb = wkv_pool.tile([P, NHT * DKV], BF16, tag="w")
            wot_sb = wow_pool.tile([P, NH * H], BF16, tag="w")

            # ---- DMAs in need-order: first wq chunks + slab0 pieces, then
            # the rest of the weights, early slabs, and WoT ----
            slabs[0] = big_pool.tile([P, NHT * P], BF16, tag="big",
                                     name="slab_0")
            # absolute minimum for the first (kv) matmul: WkvT t=0 + slab0
            # t=0; the small WkvT streams in fully while WqT follows
            hkv = NHT * DKV // 4
            nc.sync.dma_start(wkvt_sb[:, 0:hkv], wkvt.ap()[:, 0:hkv])
            nc.sync.dma_start(slabs[0][:, 0:P], hst.ap()[0][:, 0:P])
            nc.sync.dma_start(slabs[0][:, P:2 * P], hst.ap()[0][:, P:2 * P])
            for c4 in range(1, 4):
                nc.sync.dma_start(wkvt_sb[:, c4 * hkv:(c4 + 1) * hkv],
                                  wkvt.ap()[:, c4 * hkv:(c4 + 1) * hkv])
            CC = 2 * DQ  # two t-chunks of WqT per DMA
            sq = NHT * P // 4
            nc.sync.dma_start(slabs[0][:, 2 * P:sq], hst.ap()[0][:, 2 * P:sq])
            for cc in range(10):
                nc.sync.dma_start(wqt_sb[:, cc * CC:(cc + 1) * CC],
                                  wqt.ap()[:, cc * CC:(cc + 1) * CC])
                if cc < 3:
                    nc.sync.dma_start(
                        slabs[0][:, (cc + 1) * sq:(cc + 2) * sq],
                        hst.ap()[0][:, (cc + 1) * sq:(cc + 2) * sq])
                elif cc == 3:
                    tabts[0] = tab_pool.tile([P, 4 * P], BF16, tag="tab",
                                             name="tabt_0")
                    nc.sync.dma_start(tabts[0][:], tab.ap()[0])
                elif cc == 4:
                    # consts (needed from the first transpose/attn on)
                    maskb_sb = cst_pool.tile([P, P], BF16, tag="maskb")
                    nc.sync.dma_start(maskb_sb[:], maskb.ap())
                    ident_sb = cst_pool.tile([P, P], BF16, tag="ident")
                    nc.sync.dma_start(ident_sb[:], ident.ap())
                    onessq_sb = cst_pool.tile([P, P], BF16, tag="onessq")
                    nc.sync.dma_start(onessq_sb[:], onessq.ap())
                elif cc == 5:
                    load_inputs(1)
            load_inputs(2)
            load_inputs(3)
            hwo = NH * H // 4
            for cc in range(4):
                nc.sync.dma_start(wot_sb[:, cc * hwo:(cc + 1) * hwo],
                                  wot.ap()[:, cc * hwo:(cc + 1) * hwo])

            # persistent attention operands
            kT = kv_pool.tile([P, S], BF16, tag="kt")          # [d, t]
            v_sb = kv_pool.tile([P, NS, P], BF16, tag="v")     # [t, tile, d]

            qT = [None] * NCH
            attnT = [None] * NCH
            pending_tp = [None]  # deferred transpose of the previous tile

            def flush_tp():
                if pending_tp[0] is None:
                    return
                i, g, r, q_ro, k_ro = pending_tp[0]
                pending_tp[0] = None
                tp = tp_pool.tile([P, 5 * P], BF16, tag="tp", name=f"tp_{i}")
                for h in range(NH):
                    nc.tensor.transpose(
                        tp[:, h * P:(h + 1) * P], q_ro[:, h * P:(h + 1) * P],
                        ident_sb[:],
                    )
                nc.tensor.transpose(tp[:, DQ:DQ + P], k_ro[:], ident_sb[:])
                if qT[g] is None:
                    qT[g] = qa_pool.tile([P, NCH, NH, P], BF16, tag="qa",
                                         name=f"qT_{g}")
                nc.vector.tensor_copy(
                    qT[g][:, r, :, :].rearrange("p h d -> p (h d)"),
                    tp[:, 0:DQ],
                )
                nc.vector.tensor_copy(kT[:, i * P:(i + 1) * P], tp[:, DQ:DQ + P])

            proj_ps = {}

            def proj_mm(i, warm=None, warm_g=None):
                load_inputs(i + 2)
                load_inputs(i + 3)
                slab = slabs.pop(i)

                q_ps = ps_pool.tile([P, DQ], F32, tag="mm")
                kv_ps = ps_pool.tile([P, DKV], F32, tag="mm")
                # kv first for the DMA-starved opening tiles (WkvT is small
                # and lands before the WqT stream completes)
                passes = [(kv_ps, wkvt_sb, DKV), (q_ps, wqt_sb, DQ)]
                if i >= 3:
                    passes.reverse()
                for pi, (ps, wsb, dw) in enumerate(passes):
                    for t in range(NHT):
                        nc.tensor.matmul(
                            ps[:], slab[:, t * P:(t + 1) * P],
                            wsb[:, t * dw:(t + 1) * dw],
                            start=(t == 0), stop=(t == NHT - 1),
                        )
                    if pi == 0:
                        # the previous tile's transposes land between the
                        # two projection passes: its rope chain (DVE) hides
                        # under pass 1, and the qT/kT copies (DVE) hide
                        # under pass 2 — so a directly-following attention
                        # phase finds qT ready
                        flush_tp()
                proj_ps[i] = (q_ps, kv_ps)
                if warm is not None:
                    # prime the upcoming attention phase's exp pipeline
                    s_mm(warm_g, 0, 0, warm)
                    s_mm(warm_g, 0, 1, warm)

            def proj_post(i):
                g, r = i // NCH, i % NCH
                q_ps, kv_ps = proj_ps.pop(i)
                tabt = tabts.pop(i)
                cq_t, sq_t = tabt[:, 0:P], tabt[:, P:2 * P]
                ck_t, sk_t = tabt[:, 2 * P:3 * P], tabt[:, 3 * P:4 * P]

                # v evict (f32 psum -> bf16); gpsimd can't read PSUM
                nc.scalar.activation(v_sb[:, i, :], kv_ps[:, P:DKV],
                                     mybir.ActivationFunctionType.Copy)

                # ---- rms-norm stats (ACT squares, DVE rsqrt poly) ----
                q2 = wk_pool.tile([P, DQ], F32, tag="wk", name=f"q2_{i}")
                nc.scalar.activation(
                    q2[:], q_ps[:, 0:DQ],
                    mybir.ActivationFunctionType.Square,
                )
                ss = sm_pool.tile([P, NH + 1], F32, tag="ssq")
                nc.vector.tensor_reduce(
                    ss[:, 0:NH], q2[:].rearrange("p (h d) -> p h d", h=NH),
                    mybir.AxisListType.X, mybir.AluOpType.add,
                )
                junk = sm_pool.tile([P, P], F32, tag="junk")
                nc.scalar.activation(
                    junk[:], kv_ps[:, 0:P],
                    mybir.ActivationFunctionType.Square,
                    accum_out=ss[:, NH:NH + 1],
                )
                # rstd = rsqrt(ss/P) via poly+Newton on DVE (keeps ACT in
                # its exp table; Sqrt/Ln would force 1.3us table reloads).
                # Poly in raw ss: coefficients pre-divided by powers of P.
                NW = NH + 1
                h1 = sm_pool.tile([P, NW], F32, tag="h1")
                nc.vector.tensor_scalar(
                    h1[:], ss[:], RSQ_C3 / P ** 3, RSQ_C2 / P ** 2,
                    mybir.AluOpType.mult, mybir.AluOpType.add,
                )
                nc.vector.tensor_tensor(h1[:], h1[:], ss[:],
                                        mybir.AluOpType.mult)
                nc.vector.tensor_scalar_add(h1[:], h1[:], RSQ_C1 / P)
                y0 = sm_pool.tile([P, NW], F32, tag="y0")
                nc.vector.tensor_tensor(y0[:], h1[:], ss[:],
                                        mybir.AluOpType.mult)
                nc.vector.tensor_scalar_add(y0[:], y0[:], RSQ_C0)
                # Newton: rstd = y0 * (1.5 - (ss/(2P))*y0^2)
                t1 = sm_pool.tile([P, NW], F32, tag="t1")
                nc.vector.tensor_tensor(t1[:], y0[:], y0[:],
                                        mybir.AluOpType.mult)
                nc.vector.tensor_tensor(t1[:], t1[:], ss[:],
                                        mybir.AluOpType.mult)
                nc.vector.tensor_scalar(
                    t1[:], t1[:], -0.5 / P, 1.5,
                    mybir.AluOpType.mult, mybir.AluOpType.add,
                )
                rstd = sm_pool.tile([P, NW], F32, tag="rsq")
                nc.vector.tensor_tensor(rstd[:], y0[:], t1[:],
                                        mybir.AluOpType.mult)
                rstd_q, rstd_k = rstd[:, 0:NH], rstd[:, NH:NH + 1]

                # ---- fused norm-scale + rope (DVE, bf16 after first mult) ----
                def rope(ps_slice, nh, rstd, cos_t, sin_t, nm):
                    w = nh * P
                    qn = wk_pool.tile([P, w], BF16, tag="wk", name=f"qn_{nm}_{i}")
                    q3 = qn[:].rearrange("p (h d) -> p h d", h=nh)
                    nc.vector.tensor_tensor(
                        q3, ps_slice.rearrange("p (h d) -> p h d", h=nh),
                        rstd[:, :, None].broadcast_to([P, nh, P]),
                        mybir.AluOpType.mult,
                    )
                    r1 = wk_pool.tile([P, w], BF16, tag="wk", name=f"r1_{nm}_{i}")
                    nc.vector.tensor_tensor(
                        r1[:].rearrange("p (h d) -> p h d", h=nh), q3,
                        cos_t[:, None, :].broadcast_to([P, nh, P]),
                        mybir.AluOpType.mult,
                    )
                    r2 = wk_pool.tile([P, w], BF16, tag="wk", name=f"r2_{nm}_{i}")
                    r23 = r2[:].rearrange("p (h d) -> p h d", h=nh)
                    nc.vector.tensor_tensor(
                        r23[:, :, 0:64], q3[:, :, 64:P],
                        sin_t[:, None, 0:64].broadcast_to([P, nh, 64]),
                        mybir.AluOpType.mult,
                    )
                    nc.vector.tensor_tensor(
                        r23[:, :, 64:P], q3[:, :, 0:64],
                        sin_t[:, None, 64:P].broadcast_to([P, nh, 64]),
                        mybir.AluOpType.mult,
                    )
                    ro = qw_pool.tile([P, w], BF16, tag="qw", name=f"ro_{nm}_{i}")
                    nc.vector.tensor_tensor(
                        ro[:], r1[:], r2[:], mybir.AluOpType.add,
                    )
                    return ro

                q_ro = rope(q_ps[:, 0:DQ], NH, rstd_q, cq_t, sq_t, "q")
                k_ro = rope(kv_ps[:, 0:P], 1, rstd_k, ck_t, sk_t, "k")
                # transposes deferred to the next tile's matmul shadow
                pending_tp[0] = (i, g, r, q_ro, k_ro)

            def s_mm(g, h, j, sts):
                r0 = max(0, j - 4 * g)
                w = CW - r0 * P
                st = ps_pool.tile([P, CW], F32, tag="mm",
                                  name=f"st_{g}_{h}_{j}")
                nc.tensor.matmul(
                    st[:, 0:w],
                    kT[:, j * P:(j + 1) * P],
                    qT[g][:, r0:NCH, h, :],
                )
                sts[(h, j)] = st

            def attn_phase(g, hs, warm=None, look=2):
                """Attention units (heads hs) as one flat S-matmul stream
                with cross-unit lookahead, so the PE never drains at the
                unit boundary waiting for the first exp. `warm` carries S
                tiles pre-issued by the previous phase."""
                njt = 4 * g + 4  # t-tiles 0..4g+3
                seq = [(h, j) for h in hs for j in range(njt)]
                sts = warm if warm is not None else {}
                acc = {}  # h -> (av_ps, rb_ps)

                LOOK = look
                for k in range(min(LOOK, len(seq))):
                    if seq[k] not in sts:
                        s_mm(g, seq[k][0], seq[k][1], sts)
                for k, (h, j) in enumerate(seq):
                    if k + LOOK < len(seq) and seq[k + LOOK] not in sts:
                        h2, j2 = seq[k + LOOK]
                        s_mm(g, h2, j2, sts)
                    if j == 0:
                        acc[h] = (
                            ps_pool.tile([P, CW], F32, tag="mm",
                                         name=f"av_{g}_{h}"),
                            ps_pool.tile([P, CW], F32, tag="mm",
                                         name=f"rb_{g}_{h}"),
                        )
                    av_ps, rb_ps = acc[h]
                    r0 = max(0, j - 4 * g)
                    off = r0 * P
                    w = CW - off
                    st = sts.pop((h, j))
                    ptile = pt_pool.tile([P, CW], BF16, tag="pt",
                                         name=f"pt_{g}_{h}_{j}")
                    nc.scalar.activation(
                        ptile[:, 0:w], st[:, 0:w],
                        mybir.ActivationFunctionType.Exp, scale=SCALE,
                    )
                    if j >= 4 * g:
                        # zero out the masked (strictly-upper) part of the
                        # diagonal block: cheaper than -inf add pre-exp
                        nc.vector.tensor_tensor(
                            ptile[:, 0:P], ptile[:, 0:P], maskb_sb[:],
                            mybir.AluOpType.mult,
                        )
                    nc.tensor.matmul(
                        rb_ps[:, off:off + w], onessq_sb[:], ptile[:, 0:w],
                        start=(j == 0), stop=(j == njt - 1),
                    )
                    nc.tensor.matmul(
                        av_ps[:, off:off + w], v_sb[:, j, :], ptile[:, 0:w],
                        start=(j == 0), stop=(j == njt - 1),
                    )
                    if j == njt - 1:
                        # normalize: approx reciprocal of broadcast rowsums
                        recipb = wk_pool.tile([P, CW], F32, tag="wk",
                                              name=f"rc_{g}_{h}")
                        nc.vector.reciprocal_approx_fast(recipb[:], rb_ps[:])
                        if attnT[g] is None:
                            attnT[g] = qa_pool.tile([P, NH, CW], BF16,
                                                    tag="qa",
                                                    name=f"attnT_{g}")
                        nc.vector.tensor_tensor(
                            attnT[g][:, h, :], av_ps[:], recipb[:],
                            mybir.AluOpType.mult,
                        )
                        del acc[h]

            def wo_tile(i, tail=False):
                g, r = i // NCH, i % NCH
                o_stage = os_pool.tile([P, H], BF16, tag="os", name=f"ost_{i}")
                for jc in range(NJC):
                    o_ps = ps_pool.tile([P, CW], F32, tag="mm",
                                        name=f"op_{i}_{jc}")
                    for f in range(NH):
                        nc.tensor.matmul(
                            o_ps[:],
                            attnT[g][:, f, r * P:(r + 1) * P],
                            wot_sb[:, f * H + jc * CW:f * H + (jc + 1) * CW],
                            start=(f == 0), stop=(f == NH - 1),
                        )
                    # in the tail the ACT engine is co-critical with the last
                    # attention chunk's exps: keep all evicts on DVE there
                    eng = (nc.vector.tensor_copy if tail or jc % 2 == 1
                           else nc.scalar.copy)
                    eng(o_stage[:, jc * CW:(jc + 1) * CW], o_ps[:])
                    # per-jc DMA: starts draining while later jc still compute
                    nc.sync.dma_start(
                        out.ap()[i * P:(i + 1) * P, jc * CW:(jc + 1) * CW],
                        o_stage[:, jc * CW:(jc + 1) * CW])

            # ====== main schedule: proj s-tiles with attention + output
            # projection spread finely across the next chunk's tiles; the
            # attn/wo hooks sit between a tile's matmuls and its stats/rope
            # so the exp stream gets the ACT engine first ======
            warm01, warm23 = {}, {}
            for i in range(NS):
                if i >= NCH and i % NCH == 0:
                    warm01 = {}
                    proj_mm(i, warm01, (i - NCH) // NCH)
                else:
                    proj_mm(i)
                if i >= NCH:
                    g, ph = (i - NCH) // NCH, i % NCH
                    if ph == 0:
                        attn_phase(g, (0, 1), warm=warm01)
                        # pre-issue the next phase's first S matmuls so it
                        # starts with its exp pipeline already primed
                        warm23 = {}
                        s_mm(g, 2, 0, warm23)
                        s_mm(g, 2, 1, warm23)
                        proj_post(i)
                    elif ph == 1:
                        attn_phase(g, (2, 3), warm=warm23)
                        proj_post(i)
                    elif ph == 2:
                        # rope before the wo tiles: its DVE chain must be
                        # done by the next tile's transpose flush
                        proj_post(i)
                        wo_tile(NCH * g)
                        wo_tile(NCH * g + 1)
                    else:
                        proj_post(i)
                        wo_tile(NCH * g + 2)
                        wo_tile(NCH * g + 3)
                else:
                    proj_post(i)
            flush_tp()
            attn_phase(NCH - 1, (0, 1, 2, 3), look=3)
            for r in range(NCH):
                wo_tile((NCH - 1) * NCH + r, tail=True)
    nc.compile()
    return nc


def kernel(hidden_states, cos, sin, Wq, Wk, Wv, Wo, q_norm_w, k_norm_w):
    hs = np.asarray(hidden_states, dtype=np.float32)[0]      # [S, H]
    cos0 = np.asarray(cos, dtype=np.float32)[0]              # [S, 128]
    sin0 = np.asarray(sin, dtype=np.float32)[0]
    Wq = np.asarray(Wq, dtype=np.float32)
    Wk = np.asarray(Wk, dtype=np.float32)
    Wv = np.asarray(Wv, dtype=np.float32)
    Wo = np.asarray(Wo, dtype=np.float32)
    qw = np.asarray(q_norm_w, dtype=np.float32)
    kw = np.asarray(k_norm_w, dtype=np.float32)

    BF = ml_dtypes.bfloat16

    # slab[i][p][t*128+s] = hs[i*128+s, t*128+p]
    hst_t = np.ascontiguousarray(
        hs.reshape(NS, P, NHT, P).transpose(0, 3, 2, 1).reshape(NS, P, NHT * P)
    ).astype(BF)
    sgn = np.concatenate([-np.ones(64, np.float32), np.ones(64, np.float32)])

    def tables(w):
        wr = np.concatenate([w[64:], w[:64]])                # w[(i+64)%128]
        return cos0 * w[None, :], sin0 * (sgn * wr)[None, :]

    cosq_t, sinq_t = tables(qw)
    cosk_t, sink_t = tables(kw)
    tab_t = np.ascontiguousarray(
        np.concatenate([cosq_t, sinq_t, cosk_t, sink_t], axis=1)
        .astype(np.float32).reshape(NS, P, 4 * P)
    ).astype(BF)
    idx = np.arange(P)
    maskb_np = (idx[None, :] >= idx[:, None]).astype(BF)
    ident_np = np.eye(P, dtype=np.float32).astype(BF)
    onessq_np = np.ones((P, P), np.float32).astype(BF)

    if "nc" not in _CACHE:
        _CACHE["nc"] = _build()
    nc = _CACHE["nc"]

    in_maps = []
    for c in range(8):
        wq_c = Wq[c * DQ:(c + 1) * DQ, :]                    # [512, H]
        wqt_t = np.ascontiguousarray(
            wq_c.reshape(DQ, NHT, P).transpose(2, 1, 0).reshape(P, NHT * DQ)
        ).astype(BF)
        kv_c = np.concatenate([Wk[c * P:(c + 1) * P, :], Wv[c * P:(c + 1) * P, :]], axis=0)
        wkvt_t = np.ascontiguousarray(
            kv_c.reshape(DKV, NHT, P).transpose(2, 1, 0).reshape(P, NHT * DKV)
        ).astype(BF)
        wot_c = np.ascontiguousarray(Wo[:, c * DQ:(c + 1) * DQ].T)  # [512, H]
        wot_t = np.ascontiguousarray(
            wot_c.reshape(NH, P, H).transpose(1, 0, 2).reshape(P, NH * H)
        ).astype(BF)
        in_maps.append(dict(
            hst=hst_t, wqt=wqt_t, wkvt=wkvt_t, wot=wot_t, tab=tab_t,
            maskb=maskb_np, ident=ident_np, onessq=onessq_np,
        ))

    try:
        r = run_bass_kernel_spmd(nc, in_maps, core_ids=list(range(8)))
    except Exception:
        r = run_bass_kernel_spmd(nc, in_maps, core_ids=list(range(8)))
    acc = np.zeros((S, H), dtype=np.float32)
    for c in range(8):
        acc += np.asarray(r.results[c]["out"], dtype=np.float32)
    return acc[None, :, :]
